# revision 13
# baseline (speedup 1.0000x reference)
"""Self-contained Trainium2 Bass kernel for a 3-layer DGL-style GCN + NLL loss.

Strategy (8 NeuronCores, SPMD), v2:
  - Nodes re-labeled into a [chunk][core][window][128] layout: 98 windows of
    128 node slots per core (12544 slots, 12500 real).  4 chunks of 25/25/24/24
    windows double as AllGather chunking and the 4 gather sub-tables (each
    < 32768 rows so gather indices fit in int16).
  - Edges (dst-sorted) are grouped per (dst window, src chunk); each group is
    padded to 128-edge tiles.  Per tile the SpMM is a one-hot matmul
        aggT[D, n] += g[e, D].T @ S_w[e, n],  S_w[e, n] = w_e * 1[dst_e == n]
    accumulated in PSUM over the whole window.
  - v2 changes vs v1:
      * S_w is PRECOMPUTED ON HOST and streamed in via plain HWDGE DMA
        (kills ~1.7 ms of DVE one-hot construction).
      * Layer-0 h[src] rows are HOST-PERMUTED into edge order (fe stream) and
        read with sequential DMA (no gathers at all in layer 0).
      * Layer-1/2 gathers are merged into 4096-index dma_gather chunks per
        sub-table stream (SWDGE fixed cost ~1 us/instr amortized 32x),
        double-buffered through rotating SBUF tiles.
  - Dense layer: h = relu(aggT.T @ W + b) via matmul pair; layer 3 keeps
    logits in PSUM and computes the masked NLL tail on-chip; each core emits
    a partial NLL sum, host sums / N.
  - bf16 data plane, f32 PSUM accumulation and f32 softmax/NLL tail.
"""

import numpy as np
import os

N = 100000
E = 1600000
D = 128
C = 40
NCORES = 8
RPC = 12500            # real nodes per core
WPC = 98               # windows per core
PW = 128               # nodes per window
NPC = WPC * PW         # 12544 slots per core
CH_W = [25, 25, 24, 24]
CH_W0 = [0, 25, 50, 74]
CH_ROWS = [w * PW * NCORES for w in CH_W]      # rows per shared chunk region
CH_BASE = np.concatenate([[0], np.cumsum(CH_ROWS)]).astype(np.int64)
CHUNK_T = 32           # tiles per gather/DMA chunk (4096 indices)
LA = 8                 # lookahead windows for chunk issue
KG = 3                 # rotating gather buffers per stream
KSW = 4                # rotating S_w buffers

LAST_EXEC_NS = None
LAST_RESULT = None


def _chunk_of_window(w):
    for c in range(4):
        if CH_W0[c] <= w < CH_W0[c] + CH_W[c]:
            return c
    raise AssertionError(w)


CHUNK_OF_W = np.array([_chunk_of_window(w) for w in range(WPC)])
CW_ARR = np.array(CH_W)
CW0_ARR = np.array(CH_W0)


def _slot_decomp(node):
    """node id -> (core k, window w, partition p, chunk c, row-in-chunk)."""
    node = np.asarray(node, dtype=np.int64)
    k = node // RPC
    off = node % RPC
    w = off // PW
    p = off % PW
    c = CHUNK_OF_W[w]
    rowc = k * (CW_ARR[c] * PW) + (w - CW0_ARR[c]) * PW + p
    return k, w, p, c, rowc


def _wrap_idx(vals):
    """[n*128] int16 idx -> [128, n*8] wrapped (16-partition wrap, 8x rep)."""
    cap = len(vals)
    wrapped = vals.reshape(cap // 16, 16).T          # [16, cap/16]
    return np.tile(wrapped, (8, 1))                   # [128, cap/16]


def kernel(features, edge_w, W1, b1, W2, b2, W3, b3, src, dst, labels):
    import sys
    for p in ("/opt/trn_rl_repo",):
        if p not in sys.path:
            sys.path.insert(0, p)
    import ml_dtypes
    import concourse.bass as bass
    import concourse.bacc as bacc
    import concourse.mybir as mybir
    import concourse.tile as tile
    from concourse.bass_utils import run_bass_kernel_spmd

    bf16 = mybir.dt.bfloat16
    f32 = mybir.dt.float32
    i16 = mybir.dt.int16

    features = np.asarray(features, dtype=np.float32)
    edge_w = np.asarray(edge_w, dtype=np.float32)
    W1 = np.asarray(W1, dtype=np.float32); b1 = np.asarray(b1, dtype=np.float32)
    W2 = np.asarray(W2, dtype=np.float32); b2 = np.asarray(b2, dtype=np.float32)
    W3 = np.asarray(W3, dtype=np.float32); b3 = np.asarray(b3, dtype=np.float32)
    src = np.asarray(src, dtype=np.int64)
    dst = np.asarray(dst, dtype=np.int64)
    labels = np.asarray(labels, dtype=np.int64)

    # ---------------- host-side graph preprocessing ----------------
    _, _, _, src_chunk, src_rowc = _slot_decomp(src)
    _, dst_w, dst_p, _, _ = _slot_decomp(dst)

    grp = dst_w * 4 + src_chunk            # group id within a core
    NG = WPC * 4

    core_bounds = np.searchsorted(dst, np.arange(NCORES + 1) * RPC)
    cnt = np.zeros((NCORES, NG), dtype=np.int64)
    order_per_core = []
    for k in range(NCORES):
        s0, s1 = core_bounds[k], core_bounds[k + 1]
        o = np.argsort(grp[s0:s1], kind="stable") + s0
        order_per_core.append(o)
        cnt[k] = np.bincount(grp[s0:s1], minlength=NG)

    cnt_max = cnt.max(axis=0)
    Tws = -(-cnt_max // PW)                # tiles per (w,s); 0 if group empty
    Tws = Tws.reshape(WPC, 4)
    assert Tws.sum(axis=1).min() >= 1, "window with zero edges"
    TC = int(Tws.sum())                    # total tiles per layer per core

    # window-major tile offsets: order (w, s, t)
    ot_ws = np.zeros((WPC, 4), dtype=np.int64)
    pos = 0
    for w in range(WPC):
        for s in range(4):
            ot_ws[w, s] = pos
            pos += int(Tws[w, s])
    ot_end_w = np.array([ot_ws[w, 3] + Tws[w, 3] for w in range(WPC)])

    # stream-major tile offsets: for each s, order (w, t)
    spos_ws = np.zeros((WPC, 4), dtype=np.int64)
    Ts = np.zeros(4, dtype=np.int64)
    for s in range(4):
        pos = 0
        for w in range(WPC):
            spos_ws[w, s] = pos
            pos += int(Tws[w, s])
        Ts[s] = pos
    spos_end_w = np.array([[spos_ws[w, s] + Tws[w, s] for s in range(4)]
                           for w in range(WPC)])
    soff_idxcols = np.concatenate([[0], np.cumsum(Ts * 8)]).astype(np.int64)
    IC = int(soff_idxcols[-1])

    # per-core edge slot assignment + payloads
    featbf = features.astype(ml_dtypes.bfloat16)
    IDX = np.zeros((NCORES, 128, IC), dtype=np.int16)
    SW = np.zeros((NCORES, 128, TC, 128), dtype=ml_dtypes.bfloat16)
    FE = np.zeros((NCORES, 128, TC, D), dtype=ml_dtypes.bfloat16)
    for k in range(NCORES):
        o = order_per_core[k]
        e_rowc = src_rowc[o]
        e_src = src[o]
        e_dl = dst_p[o]
        e_w = edge_w[o].astype(np.float32)

        # slot positions: group (w,s) occupies tiles [ot|spos, +T), slot j ->
        # tile j//128, partition j%128
        grp_off = np.concatenate([[0], np.cumsum(cnt[k])]).astype(np.int64)
        g_sorted = grp[o]
        j_in_grp = np.arange(len(o)) - grp_off[g_sorted]
        gw = g_sorted // 4
        gs = g_sorted % 4
        tile_wm = ot_ws[gw, gs] + j_in_grp // PW
        tile_sm = spos_ws[gw, gs] + j_in_grp // PW
        part = j_in_grp % PW

        # idx stream (stream-major), padded slots stay 0
        for s in range(4):
            m = gs == s
            vals = np.zeros(int(Ts[s]) * PW, dtype=np.int16)
            vals[(tile_sm[m] * PW + part[m])] = e_rowc[m].astype(np.int16)
            IDX[k, :, soff_idxcols[s]:soff_idxcols[s + 1]] = _wrap_idx(vals)

        # S_w (window-major): [part, tile, dst_loc] = edge weight
        SW[k, part, tile_wm, e_dl] = e_w.astype(ml_dtypes.bfloat16)
        # layer-0 feature stream (window-major edge order)
        FE[k, part, tile_wm, :] = featbf[e_src]

    # labels / mask per (core, partition, window)
    LBL = np.zeros((NCORES, 128, WPC), dtype=np.float32)
    MASK = np.zeros((NCORES, 128, WPC), dtype=np.float32)
    nn = np.arange(N)
    kk = nn // RPC
    off = nn % RPC
    LBL[kk, off % PW, off // PW] = labels.astype(np.float32)
    MASK[kk, off % PW, off // PW] = 1.0

    W1b = W1.astype(ml_dtypes.bfloat16)
    W2b = W2.astype(ml_dtypes.bfloat16)
    W3b = W3.astype(ml_dtypes.bfloat16)
    B1b = b1.reshape(1, -1).astype(ml_dtypes.bfloat16)
    B2b = b2.reshape(1, -1).astype(ml_dtypes.bfloat16)
    B3b = b3.reshape(1, -1).astype(ml_dtypes.bfloat16)

    # ---------------- bass program ----------------
    nc = bacc.Bacc("TRN2", target_bir_lowering=False, debug=False,
                   num_devices=NCORES, num_swdge_queues=4)

    fe_t = nc.dram_tensor("fe", [128, TC, D], bf16, kind="ExternalInput")
    sw_t = nc.dram_tensor("sw", [128, TC, 128], bf16, kind="ExternalInput")
    idx_t = nc.dram_tensor("idx", [128, IC], i16, kind="ExternalInput")
    lbl_t = nc.dram_tensor("lbl", [128, WPC], f32, kind="ExternalInput")
    mask_t = nc.dram_tensor("mask", [128, WPC], f32, kind="ExternalInput")
    w1_t = nc.dram_tensor("w1", [D, D], bf16, kind="ExternalInput")
    w2_t = nc.dram_tensor("w2", [D, D], bf16, kind="ExternalInput")
    w3_t = nc.dram_tensor("w3", [D, C], bf16, kind="ExternalInput")
    b1_t = nc.dram_tensor("bb1", [1, D], bf16, kind="ExternalInput")
    b2_t = nc.dram_tensor("bb2", [1, D], bf16, kind="ExternalInput")
    b3_t = nc.dram_tensor("bb3", [1, C], bf16, kind="ExternalInput")
    out_t = nc.dram_tensor("out", [1, 1], f32, kind="ExternalOutput")
    dump_layer = int(os.environ.get("GCN_DUMP", "-1"))
    hd_t = None
    if dump_layer >= 0:
        hd_t = [nc.dram_tensor(f"hd{c}", [CH_W[c] * PW, D], bf16,
                               kind="ExternalOutput") for c in range(4)]

    with tile.TileContext(nc) as tc:
        with (
            tc.tile_pool(name="const", bufs=1) as cpool,
            tc.tile_pool(name="small", bufs=3) as spool,
            tc.tile_pool(name="nll", bufs=2) as npool,
            tc.tile_pool(name="ps_agg", bufs=2, space="PSUM") as ps_agg,
            tc.tile_pool(name="ps_h", bufs=2, space="PSUM") as ps_h,
            tc.tile_pool(name="dram", bufs=1, space="DRAM") as dram,
            tc.tile_pool(name="gp0", bufs=KG) as gp0,
            tc.tile_pool(name="gp1", bufs=KG) as gp1,
            tc.tile_pool(name="gp2", bufs=KG) as gp2,
            tc.tile_pool(name="gp3", bufs=KG) as gp3,
            tc.tile_pool(name="swp", bufs=KSW) as swpool,
        ):
            gpool = [gp0, gp1, gp2, gp3]
            # ---- resident metadata ----
            idx_s = cpool.tile([128, IC], i16)
            lbl_s = cpool.tile([128, WPC], f32)
            mask_s = cpool.tile([128, WPC], f32)
            nc.sync.dma_start(out=idx_s[:], in_=idx_t[:])
            nc.sync.dma_start(out=lbl_s[:], in_=lbl_t[:])
            nc.sync.dma_start(out=mask_s[:], in_=mask_t[:])
            w_s = [cpool.tile([D, D], bf16, tag="w1", name="w1s"),
                   cpool.tile([D, D], bf16, tag="w2", name="w2s"),
                   cpool.tile([D, C], bf16, tag="w3", name="w3s")]
            nc.sync.dma_start(out=w_s[0][:], in_=w1_t[:])
            nc.sync.dma_start(out=w_s[1][:], in_=w2_t[:])
            nc.sync.dma_start(out=w_s[2][:], in_=w3_t[:])
            b_s = [cpool.tile([1, D], bf16, tag="b1", name="b1s"),
                   cpool.tile([1, D], bf16, tag="b2", name="b2s"),
                   cpool.tile([1, C], bf16, tag="b3", name="b3s")]
            nc.sync.dma_start(out=b_s[0][:], in_=b1_t[:])
            nc.sync.dma_start(out=b_s[1][:], in_=b2_t[:])
            nc.sync.dma_start(out=b_s[2][:], in_=b3_t[:])

            iota40 = cpool.tile([128, C], f32)
            nc.gpsimd.iota(iota40[:], pattern=[[1, C]], base=0,
                           channel_multiplier=0,
                           allow_small_or_imprecise_dtypes=True)
            ones1 = cpool.tile([1, 128], bf16)
            nc.vector.memset(ones1[:], 1.0)
            onescol = cpool.tile([128, 1], f32)
            nc.vector.memset(onescol[:], 1.0)
            nll_acc = cpool.tile([128, 1], f32)
            nc.vector.memset(nll_acc[:], 0.0)

            # stream tiles are allocated from rotating pools per chunk (the
            # pool rotation is what gives WAR protection); these dicts map
            # chunk index -> live tile object for the consumers.
            sw_tiles = {}
            fe_tiles = {}
            g_tiles = [{}, {}, {}, {}]

            # ---- inter-layer DRAM tables ----
            h_mine = [[dram.tile([CH_W[c] * PW, D], bf16, tag=f"hm{l}{c}",
                                 name=f"hm{l}{c}")
                       for c in range(4)] for l in range(2)]
            h_full = [[dram.tile([CH_ROWS[c], D], bf16, tag=f"hf{l}{c}",
                                 name=f"hf{l}{c}", addr_space="Shared")
                       for c in range(4)] for l in range(2)]

            qcounter = [0]

            def issue_sw_chunk(c):
                T = int(min(CHUNK_T, TC - c * CHUNK_T))
                t_ = swpool.tile([128, CHUNK_T, 128], bf16, tag="sw",
                                 name=f"sw{c}")
                sw_tiles[c] = t_
                nc.sync.dma_start(out=t_[:, :T, :],
                                  in_=sw_t[:, c * CHUNK_T:c * CHUNK_T + T, :])

            def issue_fe_chunk(c):
                T = int(min(CHUNK_T, TC - c * CHUNK_T))
                s = c % 4
                t_ = gpool[s].tile([128, CHUNK_T, D], bf16, tag=f"g{s}",
                                   name=f"fe{c}")
                fe_tiles[c] = t_
                nc.sync.dma_start(out=t_[:, :T, :],
                                  in_=fe_t[:, c * CHUNK_T:c * CHUNK_T + T, :])

            MAXT_G = int(os.environ.get("GCN_MAXT", "8"))

            def issue_gather_chunk(layer, s, c):
                T = int(min(CHUNK_T, int(Ts[s]) - c * CHUNK_T))
                cb = int(soff_idxcols[s]) + c * CHUNK_T * 8
                t_ = gpool[s].tile([128, CHUNK_T, D], bf16, tag=f"g{s}",
                                   name=f"ga{layer}_{s}_{c}")
                g_tiles[s][c] = t_
                t0 = 0
                while t0 < T:
                    tn = min(MAXT_G, T - t0)
                    nc.gpsimd.dma_gather(
                        t_[:, t0:t0 + tn, :],
                        h_full[layer - 1][s][:],
                        idx_s[:, cb + t0 * 8:cb + (t0 + tn) * 8],
                        tn * PW, tn * PW, D,
                        queue_num=qcounter[0] % 4,
                    )
                    qcounter[0] += 1
                    t0 += tn

            rg = [list(range(NCORES))]
            dbg = os.environ.get("GCN_DEBUG", "")
            n_layers = {"L1": 1, "L12": 2}.get(dbg, 3)

            for layer in range(n_layers):
                swc = [0]
                fec = [0]
                gc = [0, 0, 0, 0]
                n_sw_chunks = -(-TC // CHUNK_T)
                sw_tiles.clear()
                for d_ in g_tiles:
                    d_.clear()

                for w in range(WPC):
                    wl = min(w + LA, WPC - 1)
                    target_wm = int(ot_end_w[wl])
                    while swc[0] * CHUNK_T < target_wm and swc[0] < n_sw_chunks:
                        issue_sw_chunk(swc[0]); swc[0] += 1
                    if layer == 0:
                        while fec[0] * CHUNK_T < target_wm and fec[0] < n_sw_chunks:
                            issue_fe_chunk(fec[0]); fec[0] += 1
                    else:
                        for s in range(4):
                            tgt = int(spos_end_w[wl, s])
                            nmax = -(-int(Ts[s]) // CHUNK_T)
                            while gc[s] * CHUNK_T < tgt and gc[s] < nmax:
                                issue_gather_chunk(layer, s, gc[s]); gc[s] += 1

                    # ---- SpMM for window w ----
                    tiles = [(s, t) for s in range(4)
                             for t in range(int(Tws[w, s]))]
                    agg = ps_agg.tile([128, 128], f32)
                    for i, (s, t) in enumerate(tiles):
                        wm = int(ot_ws[w, s]) + t
                        if layer == 0:
                            gap = fe_tiles[wm // CHUNK_T][:, wm % CHUNK_T, :]
                        else:
                            sp = int(spos_ws[w, s]) + t
                            gap = g_tiles[s][sp // CHUNK_T][:, sp % CHUNK_T, :]
                        swap = sw_tiles[wm // CHUNK_T][:, wm % CHUNK_T, :]
                        nc.tensor.matmul(out=agg[:], lhsT=gap, rhs=swap,
                                         start=(i == 0),
                                         stop=(i == len(tiles) - 1))

                    aggT_sb = spool.tile([128, 128], bf16, tag="aggT")
                    nc.scalar.copy(aggT_sb[:], agg[:])
                    Dout = C if layer == 2 else D
                    ph = ps_h.tile([128, Dout], f32)
                    nc.tensor.matmul(out=ph[:], lhsT=aggT_sb[:],
                                     rhs=w_s[layer][:], start=True, stop=False)
                    nc.tensor.matmul(out=ph[:], lhsT=ones1[:],
                                     rhs=b_s[layer][:], start=False, stop=True)
                    if layer < 2:
                        ht = spool.tile([128, D], bf16, tag="ht")
                        nc.scalar.activation(ht[:], ph[:],
                                             mybir.ActivationFunctionType.Relu)
                        c = int(CHUNK_OF_W[w])
                        r0 = (w - CH_W0[c]) * PW
                        nc.sync.dma_start(out=h_mine[layer][c][r0:r0 + PW, :],
                                          in_=ht[:])
                        if layer == dump_layer:
                            nc.sync.dma_start(out=hd_t[c][r0:r0 + PW, :],
                                              in_=ht[:])
                        if layer < n_layers - 1 and w == CH_W0[c] + CH_W[c] - 1:
                            nc.gpsimd.collective_compute(
                                "AllGather", mybir.AluOpType.bypass,
                                replica_groups=rg,
                                ins=[h_mine[layer][c].opt()],
                                outs=[h_full[layer][c].opt()],
                            )
                    else:
                        # fused masked-NLL tail (f32)
                        mx = npool.tile([128, 1], f32, tag="mx")
                        nc.vector.tensor_reduce(out=mx[:], in_=ph[:],
                                                axis=mybir.AxisListType.X,
                                                op=mybir.AluOpType.max)
                        negmx = npool.tile([128, 1], f32, tag="negmx")
                        nc.vector.tensor_scalar_mul(negmx[:], mx[:], -1.0)
                        expb = npool.tile([128, C], f32, tag="expb")
                        sumexp = npool.tile([128, 1], f32, tag="sumexp")
                        nc.scalar.activation(expb[:], ph[:],
                                             mybir.ActivationFunctionType.Exp,
                                             bias=negmx[:, 0:1],
                                             accum_out=sumexp[:])
                        lse = npool.tile([128, 1], f32, tag="lse")
                        nc.scalar.activation(lse[:], sumexp[:],
                                             mybir.ActivationFunctionType.Ln)
                        junk = npool.tile([128, C], f32, tag="junk")
                        picked = npool.tile([128, 1], f32, tag="picked")
                        nc.vector.scalar_tensor_tensor(
                            out=junk[:], in0=iota40[:],
                            scalar=lbl_s[:, w:w + 1],
                            in1=ph[:],
                            op0=mybir.AluOpType.is_equal,
                            op1=mybir.AluOpType.mult,
                            accum_out=picked[:])
                        t1 = npool.tile([128, 1], f32, tag="t1")
                        nc.vector.tensor_tensor(out=t1[:], in0=lse[:],
                                                in1=negmx[:],
                                                op=mybir.AluOpType.subtract)
                        t2 = npool.tile([128, 1], f32, tag="t2")
                        nc.vector.tensor_tensor(out=t2[:], in0=t1[:],
                                                in1=picked[:],
                                                op=mybir.AluOpType.subtract)
                        nc.vector.scalar_tensor_tensor(
                            out=nll_acc[:], in0=t2[:],
                            scalar=mask_s[:, w:w + 1],
                            in1=nll_acc[:],
                            op0=mybir.AluOpType.mult,
                            op1=mybir.AluOpType.add)

            # ---------------- final partial-sum ----------------
            pscalar = ps_h.tile([1, 1], f32, tag="pscalar")
            nc.tensor.matmul(out=pscalar[:], lhsT=nll_acc[:], rhs=onescol[:],
                             start=True, stop=True)
            res_sb = spool.tile([1, 1], f32, tag="res")
            nc.scalar.copy(res_sb[:], pscalar[:])
            nc.sync.dma_start(out=out_t[:], in_=res_sb[:])

    nc.compile()

    in_maps = []
    for k in range(NCORES):
        in_maps.append({
            "fe": FE[k], "sw": SW[k], "idx": IDX[k],
            "lbl": LBL[k], "mask": MASK[k],
            "w1": W1b, "w2": W2b, "w3": W3b,
            "bb1": B1b, "bb2": B2b, "bb3": B3b,
        })
    trace_ok = False
    try:
        from antenv.axon_hooks import get_axon_ntff_profile_hook
        trace_ok = get_axon_ntff_profile_hook() is not None
    except Exception:
        pass
    res = run_bass_kernel_spmd(nc, in_maps, list(range(NCORES)), trace=trace_ok)
    global LAST_EXEC_NS, LAST_RESULT
    LAST_EXEC_NS = res.exec_time_ns
    LAST_RESULT = res
    total = sum(float(res.results[k]["out"][0, 0]) for k in range(NCORES))
    return np.float32(total / N)


# revision 17
# speedup vs baseline: 1.0060x; 1.0060x over previous
"""Self-contained Trainium2 Bass kernel for a 3-layer DGL-style GCN + NLL loss.

Strategy (8 NeuronCores, SPMD), v2:
  - Nodes re-labeled into a [chunk][core][window][128] layout: 98 windows of
    128 node slots per core (12544 slots, 12500 real).  4 chunks of 25/25/24/24
    windows double as AllGather chunking and the 4 gather sub-tables (each
    < 32768 rows so gather indices fit in int16).
  - Edges (dst-sorted) are grouped per (dst window, src chunk); each group is
    padded to 128-edge tiles.  Per tile the SpMM is a one-hot matmul
        aggT[D, n] += g[e, D].T @ S_w[e, n],  S_w[e, n] = w_e * 1[dst_e == n]
    accumulated in PSUM over the whole window.
  - v2 changes vs v1:
      * S_w is PRECOMPUTED ON HOST and streamed in via plain HWDGE DMA
        (kills ~1.7 ms of DVE one-hot construction).
      * Layer-0 h[src] rows are HOST-PERMUTED into edge order (fe stream) and
        read with sequential DMA (no gathers at all in layer 0).
      * Layer-1/2 gathers are merged into 4096-index dma_gather chunks per
        sub-table stream (SWDGE fixed cost ~1 us/instr amortized 32x),
        double-buffered through rotating SBUF tiles.
  - Dense layer: h = relu(aggT.T @ W + b) via matmul pair; layer 3 keeps
    logits in PSUM and computes the masked NLL tail on-chip; each core emits
    a partial NLL sum, host sums / N.
  - bf16 data plane, f32 PSUM accumulation and f32 softmax/NLL tail.
"""

import numpy as np
import os

N = 100000
E = 1600000
D = 128
C = 40
NCORES = 8
RPC = 12500            # real nodes per core
WPC = 98               # windows per core
PW = 128               # nodes per window
NPC = WPC * PW         # 12544 slots per core
CH_W = [25, 25, 24, 24]
CH_W0 = [0, 25, 50, 74]
CH_ROWS = [w * PW * NCORES for w in CH_W]      # rows per shared chunk region
CH_BASE = np.concatenate([[0], np.cumsum(CH_ROWS)]).astype(np.int64)
CHUNK_T = 32           # tiles per gather/DMA chunk (4096 indices)
LA = int(os.environ.get("GCN_LA", "16"))  # lookahead windows
AG_DELAY = 4           # windows between chunk end and its AllGather issue
KG = int(os.environ.get("GCN_KG", "4"))   # gather buffers per stream
KSW = 4                # rotating S_w buffers

LAST_EXEC_NS = None
LAST_RESULT = None


def _chunk_of_window(w):
    for c in range(4):
        if CH_W0[c] <= w < CH_W0[c] + CH_W[c]:
            return c
    raise AssertionError(w)


CHUNK_OF_W = np.array([_chunk_of_window(w) for w in range(WPC)])
CW_ARR = np.array(CH_W)
CW0_ARR = np.array(CH_W0)


def _slot_decomp(node):
    """node id -> (core k, window w, partition p, chunk c, row-in-chunk)."""
    node = np.asarray(node, dtype=np.int64)
    k = node // RPC
    off = node % RPC
    w = off // PW
    p = off % PW
    c = CHUNK_OF_W[w]
    rowc = k * (CW_ARR[c] * PW) + (w - CW0_ARR[c]) * PW + p
    return k, w, p, c, rowc


def _wrap_idx(vals):
    """[n*128] int16 idx -> [128, n*8] wrapped (16-partition wrap, 8x rep)."""
    cap = len(vals)
    wrapped = vals.reshape(cap // 16, 16).T          # [16, cap/16]
    return np.tile(wrapped, (8, 1))                   # [128, cap/16]


def kernel(features, edge_w, W1, b1, W2, b2, W3, b3, src, dst, labels):
    import sys
    for p in ("/opt/trn_rl_repo",):
        if p not in sys.path:
            sys.path.insert(0, p)
    import ml_dtypes
    import concourse.bass as bass
    import concourse.bacc as bacc
    import concourse.mybir as mybir
    import concourse.tile as tile
    from concourse.bass_utils import run_bass_kernel_spmd

    bf16 = mybir.dt.bfloat16
    f32 = mybir.dt.float32
    i16 = mybir.dt.int16

    features = np.asarray(features, dtype=np.float32)
    edge_w = np.asarray(edge_w, dtype=np.float32)
    W1 = np.asarray(W1, dtype=np.float32); b1 = np.asarray(b1, dtype=np.float32)
    W2 = np.asarray(W2, dtype=np.float32); b2 = np.asarray(b2, dtype=np.float32)
    W3 = np.asarray(W3, dtype=np.float32); b3 = np.asarray(b3, dtype=np.float32)
    src = np.asarray(src, dtype=np.int64)
    dst = np.asarray(dst, dtype=np.int64)
    labels = np.asarray(labels, dtype=np.int64)

    # ---------------- host-side graph preprocessing ----------------
    _, _, _, src_chunk, src_rowc = _slot_decomp(src)
    _, dst_w, dst_p, _, _ = _slot_decomp(dst)

    grp = dst_w * 4 + src_chunk            # group id within a core
    NG = WPC * 4

    core_bounds = np.searchsorted(dst, np.arange(NCORES + 1) * RPC)
    cnt = np.zeros((NCORES, NG), dtype=np.int64)
    order_per_core = []
    for k in range(NCORES):
        s0, s1 = core_bounds[k], core_bounds[k + 1]
        # sort by (group, src row): src-sorted idx within each group gives the
        # DMA engines address-adjacent gather descriptors to aggregate
        o = np.lexsort((src_rowc[s0:s1], grp[s0:s1])) + s0
        order_per_core.append(o)
        cnt[k] = np.bincount(grp[s0:s1], minlength=NG)

    cnt_max = cnt.max(axis=0)
    Tws = -(-cnt_max // PW)                # tiles per (w,s); 0 if group empty
    Tws = Tws.reshape(WPC, 4)
    assert Tws.sum(axis=1).min() >= 1, "window with zero edges"
    TC = int(Tws.sum())                    # total tiles per layer per core

    # window-major tile offsets: order (w, s, t)
    ot_ws = np.zeros((WPC, 4), dtype=np.int64)
    pos = 0
    for w in range(WPC):
        for s in range(4):
            ot_ws[w, s] = pos
            pos += int(Tws[w, s])
    ot_end_w = np.array([ot_ws[w, 3] + Tws[w, 3] for w in range(WPC)])

    # stream-major tile offsets: for each s, order (w, t)
    spos_ws = np.zeros((WPC, 4), dtype=np.int64)
    Ts = np.zeros(4, dtype=np.int64)
    for s in range(4):
        pos = 0
        for w in range(WPC):
            spos_ws[w, s] = pos
            pos += int(Tws[w, s])
        Ts[s] = pos
    spos_end_w = np.array([[spos_ws[w, s] + Tws[w, s] for s in range(4)]
                           for w in range(WPC)])
    soff_idxcols = np.concatenate([[0], np.cumsum(Ts * 8)]).astype(np.int64)
    IC = int(soff_idxcols[-1])

    # per-core edge slot assignment + payloads
    sw_fp8 = os.environ.get("GCN_SWDT", "fp8") == "fp8"
    sw_np_dt = ml_dtypes.float8_e4m3 if sw_fp8 else ml_dtypes.bfloat16
    featbf = features.astype(ml_dtypes.bfloat16)
    IDX = np.zeros((NCORES, 128, IC), dtype=np.int16)
    SW = np.zeros((NCORES, 128, TC, 128), dtype=sw_np_dt)
    FE = np.zeros((NCORES, 128, TC, D), dtype=ml_dtypes.bfloat16)
    for k in range(NCORES):
        o = order_per_core[k]
        e_rowc = src_rowc[o]
        e_src = src[o]
        e_dl = dst_p[o]
        e_w = edge_w[o].astype(np.float32)

        # slot positions: group (w,s) occupies tiles [ot|spos, +T), slot j ->
        # tile j//128, partition j%128
        grp_off = np.concatenate([[0], np.cumsum(cnt[k])]).astype(np.int64)
        g_sorted = grp[o]
        j_in_grp = np.arange(len(o)) - grp_off[g_sorted]
        gw = g_sorted // 4
        gs = g_sorted % 4
        tile_wm = ot_ws[gw, gs] + j_in_grp // PW
        tile_sm = spos_ws[gw, gs] + j_in_grp // PW
        part = j_in_grp % PW

        # idx stream (stream-major): real edges, then 0-pads up to the group's
        # cnt_max, then -1 (skipped when trailing in a gather) up to tile cap
        for s in range(4):
            m = gs == s
            vals = np.zeros(int(Ts[s]) * PW, dtype=np.int16)
            for w in range(WPC):
                cm = int(cnt_max[w * 4 + s])
                cap = int(Tws[w, s]) * PW
                sp0 = int(spos_ws[w, s]) * PW
                vals[sp0 + cm:sp0 + cap] = -1
            vals[(tile_sm[m] * PW + part[m])] = e_rowc[m].astype(np.int16)
            IDX[k, :, soff_idxcols[s]:soff_idxcols[s + 1]] = _wrap_idx(vals)

        # S_w (window-major): [part, tile, dst_loc] = edge weight
        SW[k, part, tile_wm, e_dl] = e_w.astype(sw_np_dt)
        # layer-0 feature stream (window-major edge order)
        FE[k, part, tile_wm, :] = featbf[e_src]

    # labels / mask per (core, partition, window)
    LBL = np.zeros((NCORES, 128, WPC), dtype=np.float32)
    MASK = np.zeros((NCORES, 128, WPC), dtype=np.float32)
    nn = np.arange(N)
    kk = nn // RPC
    off = nn % RPC
    LBL[kk, off % PW, off // PW] = labels.astype(np.float32)
    MASK[kk, off % PW, off // PW] = 1.0

    W1b = W1.astype(ml_dtypes.bfloat16)
    W2b = W2.astype(ml_dtypes.bfloat16)
    W3b = W3.astype(ml_dtypes.bfloat16)
    B1b = b1.reshape(1, -1).astype(ml_dtypes.bfloat16)
    B2b = b2.reshape(1, -1).astype(ml_dtypes.bfloat16)
    B3b = b3.reshape(1, -1).astype(ml_dtypes.bfloat16)

    # ---------------- bass program ----------------
    nc = bacc.Bacc("TRN2", target_bir_lowering=False, debug=False,
                   num_devices=NCORES, num_swdge_queues=4)

    sw_dt = mybir.dt.float8e4 if sw_fp8 else bf16
    fe_t = nc.dram_tensor("fe", [128, TC, D], bf16, kind="ExternalInput")
    sw_t = nc.dram_tensor("sw", [128, TC, 128], sw_dt, kind="ExternalInput")
    idx_t = nc.dram_tensor("idx", [128, IC], i16, kind="ExternalInput")
    lbl_t = nc.dram_tensor("lbl", [128, WPC], f32, kind="ExternalInput")
    mask_t = nc.dram_tensor("mask", [128, WPC], f32, kind="ExternalInput")
    w1_t = nc.dram_tensor("w1", [D, D], bf16, kind="ExternalInput")
    w2_t = nc.dram_tensor("w2", [D, D], bf16, kind="ExternalInput")
    w3_t = nc.dram_tensor("w3", [D, C], bf16, kind="ExternalInput")
    b1_t = nc.dram_tensor("bb1", [1, D], bf16, kind="ExternalInput")
    b2_t = nc.dram_tensor("bb2", [1, D], bf16, kind="ExternalInput")
    b3_t = nc.dram_tensor("bb3", [1, C], bf16, kind="ExternalInput")
    out_t = nc.dram_tensor("out", [1, 1], f32, kind="ExternalOutput")
    dump_layer = int(os.environ.get("GCN_DUMP", "-1"))
    hd_t = None
    if dump_layer >= 0:
        hd_t = [nc.dram_tensor(f"hd{c}", [CH_W[c] * PW, D], bf16,
                               kind="ExternalOutput") for c in range(4)]

    with tile.TileContext(nc) as tc:
        with (
            tc.tile_pool(name="const", bufs=1) as cpool,
            tc.tile_pool(name="small", bufs=3) as spool,
            tc.tile_pool(name="nll", bufs=2) as npool,
            tc.tile_pool(name="ps_agg", bufs=2, space="PSUM") as ps_agg,
            tc.tile_pool(name="ps_h", bufs=2, space="PSUM") as ps_h,
            tc.tile_pool(name="dram", bufs=1, space="DRAM") as dram,
            tc.tile_pool(name="gp0", bufs=KG) as gp0,
            tc.tile_pool(name="gp1", bufs=KG) as gp1,
            tc.tile_pool(name="gp2", bufs=KG) as gp2,
            tc.tile_pool(name="gp3", bufs=KG) as gp3,
            tc.tile_pool(name="swp", bufs=KSW) as swpool,
        ):
            gpool = [gp0, gp1, gp2, gp3]
            # ---- resident metadata ----
            idx_s = cpool.tile([128, IC], i16)
            lbl_s = cpool.tile([128, WPC], f32)
            mask_s = cpool.tile([128, WPC], f32)
            nc.sync.dma_start(out=idx_s[:], in_=idx_t[:])
            nc.sync.dma_start(out=lbl_s[:], in_=lbl_t[:])
            nc.sync.dma_start(out=mask_s[:], in_=mask_t[:])
            w_s = [cpool.tile([D, D], bf16, tag="w1", name="w1s"),
                   cpool.tile([D, D], bf16, tag="w2", name="w2s"),
                   cpool.tile([D, C], bf16, tag="w3", name="w3s")]
            nc.sync.dma_start(out=w_s[0][:], in_=w1_t[:])
            nc.sync.dma_start(out=w_s[1][:], in_=w2_t[:])
            nc.sync.dma_start(out=w_s[2][:], in_=w3_t[:])
            b_s = [cpool.tile([1, D], bf16, tag="b1", name="b1s"),
                   cpool.tile([1, D], bf16, tag="b2", name="b2s"),
                   cpool.tile([1, C], bf16, tag="b3", name="b3s")]
            nc.sync.dma_start(out=b_s[0][:], in_=b1_t[:])
            nc.sync.dma_start(out=b_s[1][:], in_=b2_t[:])
            nc.sync.dma_start(out=b_s[2][:], in_=b3_t[:])

            iota40 = cpool.tile([128, C], f32)
            nc.gpsimd.iota(iota40[:], pattern=[[1, C]], base=0,
                           channel_multiplier=0,
                           allow_small_or_imprecise_dtypes=True)
            ones1 = cpool.tile([1, 128], bf16)
            nc.vector.memset(ones1[:], 1.0)
            onescol = cpool.tile([128, 1], f32)
            nc.vector.memset(onescol[:], 1.0)
            nll_acc = cpool.tile([128, 1], f32)
            nc.vector.memset(nll_acc[:], 0.0)

            # stream tiles are allocated from rotating pools per chunk (the
            # pool rotation is what gives WAR protection); these dicts map
            # chunk index -> live tile object for the consumers.
            sw_tiles = {}
            fe_tiles = {}
            g_tiles = [{}, {}, {}, {}]

            # ---- inter-layer DRAM tables ----
            h_mine = [[dram.tile([CH_W[c] * PW, D], bf16, tag=f"hm{l}{c}",
                                 name=f"hm{l}{c}")
                       for c in range(4)] for l in range(2)]
            h_full = [[dram.tile([CH_ROWS[c], D], bf16, tag=f"hf{l}{c}",
                                 name=f"hf{l}{c}", addr_space="Shared")
                       for c in range(4)] for l in range(2)]

            qcounter = [0]

            def issue_sw_chunk(c):
                T = int(min(CHUNK_T, TC - c * CHUNK_T))
                t_ = swpool.tile([128, CHUNK_T, 128], sw_dt, tag="sw",
                                 name=f"sw{c}")
                sw_tiles[c] = t_
                nc.sync.dma_start(out=t_[:, :T, :],
                                  in_=sw_t[:, c * CHUNK_T:c * CHUNK_T + T, :])

            def issue_fe_chunk(c):
                T = int(min(CHUNK_T, TC - c * CHUNK_T))
                s = c % 4
                t_ = gpool[s].tile([128, CHUNK_T, D], bf16, tag=f"g{s}",
                                   name=f"fe{c}")
                fe_tiles[c] = t_
                nc.sync.dma_start(out=t_[:, :T, :],
                                  in_=fe_t[:, c * CHUNK_T:c * CHUNK_T + T, :])

            MAXT_G = int(os.environ.get("GCN_MAXT", "8"))
            # per-stream list of (group start tile, end tile, cnt_max) in
            # stream-major order; gathers are issued per group segment so the
            # trailing -1 idx pads are actually skipped by the ucode
            grp_segs = [[] for _ in range(4)]
            for s_ in range(4):
                for w_ in range(WPC):
                    T_ = int(Tws[w_, s_])
                    if T_ == 0:
                        continue
                    g0 = int(spos_ws[w_, s_])
                    grp_segs[s_].append((g0, g0 + T_, int(cnt_max[w_ * 4 + s_])))

            def issue_gather_chunk(layer, s, c):
                lo, hi = c * CHUNK_T, min((c + 1) * CHUNK_T, int(Ts[s]))
                t_ = gpool[s].tile([128, CHUNK_T, D], bf16, tag=f"g{s}",
                                   name=f"ga{layer}_{s}_{c}")
                g_tiles[s][c] = t_
                for (g0, g1, cm) in grp_segs[s]:
                    if g1 <= lo or g0 >= hi:
                        continue
                    a, b = max(g0, lo), min(g1, hi)
                    while a < b:
                        e = min(a + MAXT_G, b)
                        nvalid = max(0, min(cm - (a - g0) * PW, (e - a) * PW))
                        cb = int(soff_idxcols[s]) + a * 8
                        nc.gpsimd.dma_gather(
                            t_[:, a - lo:e - lo, :],
                            h_full[layer - 1][s][:],
                            idx_s[:, cb:cb + (e - a) * 8],
                            (e - a) * PW, nvalid, D,
                            queue_num=qcounter[0] % 4,
                        )
                        qcounter[0] += 1
                        a = e

            rg = [list(range(NCORES))]
            dbg = os.environ.get("GCN_DEBUG", "")
            n_layers = {"L1": 1, "L12": 2}.get(dbg, 3)

            for layer in range(n_layers):
                swc = [0]
                fec = [0]
                gc = [0, 0, 0, 0]
                n_sw_chunks = -(-TC // CHUNK_T)
                sw_tiles.clear()
                for d_ in g_tiles:
                    d_.clear()

                def issue_ag(c):
                    nc.gpsimd.collective_compute(
                        "AllGather", mybir.AluOpType.bypass,
                        replica_groups=rg,
                        ins=[h_mine[layer][c].opt()],
                        outs=[h_full[layer][c].opt()],
                    )

                for w in range(WPC):
                    if layer < n_layers - 1 and layer < 2:
                        for c_ in range(4):
                            if w == CH_W0[c_] + CH_W[c_] - 1 + AG_DELAY:
                                issue_ag(c_)
                    wl = min(w + LA, WPC - 1)
                    target_wm = int(ot_end_w[wl])
                    while swc[0] * CHUNK_T < target_wm and swc[0] < n_sw_chunks:
                        issue_sw_chunk(swc[0]); swc[0] += 1
                    if layer == 0:
                        while fec[0] * CHUNK_T < target_wm and fec[0] < n_sw_chunks:
                            issue_fe_chunk(fec[0]); fec[0] += 1
                    else:
                        for s in range(4):
                            tgt = int(spos_end_w[wl, s])
                            nmax = -(-int(Ts[s]) // CHUNK_T)
                            while gc[s] * CHUNK_T < tgt and gc[s] < nmax:
                                issue_gather_chunk(layer, s, gc[s]); gc[s] += 1

                    # ---- SpMM for window w ----
                    tiles = [(s, t) for s in range(4)
                             for t in range(int(Tws[w, s]))]
                    agg = ps_agg.tile([128, 128], f32)
                    for i, (s, t) in enumerate(tiles):
                        wm = int(ot_ws[w, s]) + t
                        if layer == 0:
                            gap = fe_tiles[wm // CHUNK_T][:, wm % CHUNK_T, :]
                        else:
                            sp = int(spos_ws[w, s]) + t
                            gap = g_tiles[s][sp // CHUNK_T][:, sp % CHUNK_T, :]
                        swap = sw_tiles[wm // CHUNK_T][:, wm % CHUNK_T, :]
                        nc.tensor.matmul(out=agg[:], lhsT=gap, rhs=swap,
                                         start=(i == 0),
                                         stop=(i == len(tiles) - 1))

                    aggT_sb = spool.tile([128, 128], bf16, tag="aggT")
                    nc.scalar.copy(aggT_sb[:], agg[:])
                    Dout = C if layer == 2 else D
                    ph = ps_h.tile([128, Dout], f32)
                    nc.tensor.matmul(out=ph[:], lhsT=aggT_sb[:],
                                     rhs=w_s[layer][:], start=True, stop=False)
                    nc.tensor.matmul(out=ph[:], lhsT=ones1[:],
                                     rhs=b_s[layer][:], start=False, stop=True)
                    if layer < 2:
                        ht = spool.tile([128, D], bf16, tag="ht")
                        nc.scalar.activation(ht[:], ph[:],
                                             mybir.ActivationFunctionType.Relu)
                        c = int(CHUNK_OF_W[w])
                        r0 = (w - CH_W0[c]) * PW
                        nc.sync.dma_start(out=h_mine[layer][c][r0:r0 + PW, :],
                                          in_=ht[:])
                        if layer == dump_layer:
                            nc.sync.dma_start(out=hd_t[c][r0:r0 + PW, :],
                                              in_=ht[:])

                    else:
                        # fused masked-NLL tail (f32)
                        mx = npool.tile([128, 1], f32, tag="mx")
                        nc.vector.tensor_reduce(out=mx[:], in_=ph[:],
                                                axis=mybir.AxisListType.X,
                                                op=mybir.AluOpType.max)
                        negmx = npool.tile([128, 1], f32, tag="negmx")
                        nc.vector.tensor_scalar_mul(negmx[:], mx[:], -1.0)
                        expb = npool.tile([128, C], f32, tag="expb")
                        sumexp = npool.tile([128, 1], f32, tag="sumexp")
                        nc.scalar.activation(expb[:], ph[:],
                                             mybir.ActivationFunctionType.Exp,
                                             bias=negmx[:, 0:1],
                                             accum_out=sumexp[:])
                        lse = npool.tile([128, 1], f32, tag="lse")
                        nc.scalar.activation(lse[:], sumexp[:],
                                             mybir.ActivationFunctionType.Ln)
                        junk = npool.tile([128, C], f32, tag="junk")
                        picked = npool.tile([128, 1], f32, tag="picked")
                        nc.vector.scalar_tensor_tensor(
                            out=junk[:], in0=iota40[:],
                            scalar=lbl_s[:, w:w + 1],
                            in1=ph[:],
                            op0=mybir.AluOpType.is_equal,
                            op1=mybir.AluOpType.mult,
                            accum_out=picked[:])
                        t1 = npool.tile([128, 1], f32, tag="t1")
                        nc.vector.tensor_tensor(out=t1[:], in0=lse[:],
                                                in1=negmx[:],
                                                op=mybir.AluOpType.subtract)
                        t2 = npool.tile([128, 1], f32, tag="t2")
                        nc.vector.tensor_tensor(out=t2[:], in0=t1[:],
                                                in1=picked[:],
                                                op=mybir.AluOpType.subtract)
                        nc.vector.scalar_tensor_tensor(
                            out=nll_acc[:], in0=t2[:],
                            scalar=mask_s[:, w:w + 1],
                            in1=nll_acc[:],
                            op0=mybir.AluOpType.mult,
                            op1=mybir.AluOpType.add)

                if layer < n_layers - 1 and layer < 2:
                    for c_ in range(4):
                        if CH_W0[c_] + CH_W[c_] - 1 + AG_DELAY > WPC - 1:
                            issue_ag(c_)

            # ---------------- final partial-sum ----------------
            pscalar = ps_h.tile([1, 1], f32, tag="pscalar")
            nc.tensor.matmul(out=pscalar[:], lhsT=nll_acc[:], rhs=onescol[:],
                             start=True, stop=True)
            res_sb = spool.tile([1, 1], f32, tag="res")
            nc.scalar.copy(res_sb[:], pscalar[:])
            nc.sync.dma_start(out=out_t[:], in_=res_sb[:])

    nc.compile()

    in_maps = []
    for k in range(NCORES):
        in_maps.append({
            "fe": FE[k], "sw": SW[k], "idx": IDX[k],
            "lbl": LBL[k], "mask": MASK[k],
            "w1": W1b, "w2": W2b, "w3": W3b,
            "bb1": B1b, "bb2": B2b, "bb3": B3b,
        })
    trace_ok = False
    try:
        from antenv.axon_hooks import get_axon_ntff_profile_hook
        trace_ok = get_axon_ntff_profile_hook() is not None
    except Exception:
        pass
    res = run_bass_kernel_spmd(nc, in_maps, list(range(NCORES)), trace=trace_ok)
    global LAST_EXEC_NS, LAST_RESULT
    LAST_EXEC_NS = res.exec_time_ns
    LAST_RESULT = res
    total = sum(float(res.results[k]["out"][0, 0]) for k in range(NCORES))
    return np.float32(total / N)


# revision 22
# speedup vs baseline: 1.2848x; 1.2771x over previous
"""Self-contained Trainium2 Bass kernel for a 3-layer DGL-style GCN + NLL loss.

Strategy (8 NeuronCores, SPMD), v2:
  - Nodes re-labeled into a [chunk][core][window][128] layout: 98 windows of
    128 node slots per core (12544 slots, 12500 real).  4 chunks of 25/25/24/24
    windows double as AllGather chunking and the 4 gather sub-tables (each
    < 32768 rows so gather indices fit in int16).
  - Edges (dst-sorted) are grouped per (dst window, src chunk); each group is
    padded to 128-edge tiles.  Per tile the SpMM is a one-hot matmul
        aggT[D, n] += g[e, D].T @ S_w[e, n],  S_w[e, n] = w_e * 1[dst_e == n]
    accumulated in PSUM over the whole window.
  - v2 changes vs v1:
      * S_w is PRECOMPUTED ON HOST and streamed in via plain HWDGE DMA
        (kills ~1.7 ms of DVE one-hot construction).
      * Layer-0 h[src] rows are HOST-PERMUTED into edge order (fe stream) and
        read with sequential DMA (no gathers at all in layer 0).
      * Layer-1/2 gathers are merged into 4096-index dma_gather chunks per
        sub-table stream (SWDGE fixed cost ~1 us/instr amortized 32x),
        double-buffered through rotating SBUF tiles.
  - Dense layer: h = relu(aggT.T @ W + b) via matmul pair; layer 3 keeps
    logits in PSUM and computes the masked NLL tail on-chip; each core emits
    a partial NLL sum, host sums / N.
  - bf16 data plane, f32 PSUM accumulation and f32 softmax/NLL tail.
"""

import numpy as np
import os

N = 100000
E = 1600000
D = 128
C = 40
NCORES = 8
RPC = 12500            # real nodes per core
WPC = 98               # windows per core
PW = 128               # nodes per window
NPC = WPC * PW         # 12544 slots per core
CH_W = [25, 25, 24, 24]
CH_W0 = [0, 25, 50, 74]
CH_ROWS = [w * PW * NCORES for w in CH_W]      # rows per shared chunk region
CH_BASE = np.concatenate([[0], np.cumsum(CH_ROWS)]).astype(np.int64)
CHUNK_T = 32           # tiles per gather/DMA chunk (4096 indices)
LA = int(os.environ.get("GCN_LA", "16"))  # lookahead windows
AG_DELAY = 4           # windows between chunk end and its AllGather issue
KG = int(os.environ.get("GCN_KG", "4"))   # gather buffers per stream
KSW = 4                # rotating S_w buffers

LAST_EXEC_NS = None
LAST_RESULT = None


def _chunk_of_window(w):
    for c in range(4):
        if CH_W0[c] <= w < CH_W0[c] + CH_W[c]:
            return c
    raise AssertionError(w)


CHUNK_OF_W = np.array([_chunk_of_window(w) for w in range(WPC)])
CW_ARR = np.array(CH_W)
CW0_ARR = np.array(CH_W0)


def _slot_decomp(node):
    """node id -> (core k, window w, partition p, chunk c, row-in-chunk)."""
    node = np.asarray(node, dtype=np.int64)
    k = node // RPC
    off = node % RPC
    w = off // PW
    p = off % PW
    c = CHUNK_OF_W[w]
    rowc = k * (CW_ARR[c] * PW) + (w - CW0_ARR[c]) * PW + p
    return k, w, p, c, rowc


def _wrap_idx(vals):
    """[n*128] int16 idx -> [128, n*8] wrapped (16-partition wrap, 8x rep)."""
    cap = len(vals)
    wrapped = vals.reshape(cap // 16, 16).T          # [16, cap/16]
    return np.tile(wrapped, (8, 1))                   # [128, cap/16]


def kernel(features, edge_w, W1, b1, W2, b2, W3, b3, src, dst, labels):
    import sys
    for p in ("/opt/trn_rl_repo",):
        if p not in sys.path:
            sys.path.insert(0, p)
    import ml_dtypes
    import concourse.bass as bass
    import concourse.bacc as bacc
    import concourse.mybir as mybir
    import concourse.tile as tile
    from concourse.bass_utils import run_bass_kernel_spmd

    bf16 = mybir.dt.bfloat16
    f32 = mybir.dt.float32
    i16 = mybir.dt.int16

    features = np.asarray(features, dtype=np.float32)
    edge_w = np.asarray(edge_w, dtype=np.float32)
    W1 = np.asarray(W1, dtype=np.float32); b1 = np.asarray(b1, dtype=np.float32)
    W2 = np.asarray(W2, dtype=np.float32); b2 = np.asarray(b2, dtype=np.float32)
    W3 = np.asarray(W3, dtype=np.float32); b3 = np.asarray(b3, dtype=np.float32)
    src = np.asarray(src, dtype=np.int64)
    dst = np.asarray(dst, dtype=np.int64)
    labels = np.asarray(labels, dtype=np.int64)

    # ---------------- host-side graph preprocessing ----------------
    _, _, _, src_chunk, src_rowc = _slot_decomp(src)
    _, dst_w, dst_p, _, _ = _slot_decomp(dst)

    grp = dst_w * 4 + src_chunk            # group id within a core
    NG = WPC * 4

    core_bounds = np.searchsorted(dst, np.arange(NCORES + 1) * RPC)
    cnt = np.zeros((NCORES, NG), dtype=np.int64)
    order_per_core = []
    for k in range(NCORES):
        s0, s1 = core_bounds[k], core_bounds[k + 1]
        # sort by (group, src row): src-sorted idx within each group gives the
        # DMA engines address-adjacent gather descriptors to aggregate
        o = np.lexsort((src_rowc[s0:s1], grp[s0:s1])) + s0
        order_per_core.append(o)
        cnt[k] = np.bincount(grp[s0:s1], minlength=NG)

    cnt_max = cnt.max(axis=0)
    Tws = -(-cnt_max // PW)                # tiles per (w,s); 0 if group empty
    Tws = Tws.reshape(WPC, 4)
    assert Tws.sum(axis=1).min() >= 1, "window with zero edges"
    TC = int(Tws.sum())                    # total tiles per layer per core

    # window-major tile offsets: order (w, s, t)
    ot_ws = np.zeros((WPC, 4), dtype=np.int64)
    pos = 0
    for w in range(WPC):
        for s in range(4):
            ot_ws[w, s] = pos
            pos += int(Tws[w, s])
    ot_end_w = np.array([ot_ws[w, 3] + Tws[w, 3] for w in range(WPC)])

    # stream-major tile offsets: for each s, order (w, t)
    spos_ws = np.zeros((WPC, 4), dtype=np.int64)
    Ts = np.zeros(4, dtype=np.int64)
    for s in range(4):
        pos = 0
        for w in range(WPC):
            spos_ws[w, s] = pos
            pos += int(Tws[w, s])
        Ts[s] = pos
    spos_end_w = np.array([[spos_ws[w, s] + Tws[w, s] for s in range(4)]
                           for w in range(WPC)])
    soff_idxcols = np.concatenate([[0], np.cumsum(Ts * 8)]).astype(np.int64)
    IC = int(soff_idxcols[-1])

    # per-core edge slot assignment + payloads
    sw_fp8 = os.environ.get("GCN_SWDT", "fp8") == "fp8"
    sw_np_dt = ml_dtypes.float8_e4m3 if sw_fp8 else ml_dtypes.bfloat16
    featbf = features.astype(ml_dtypes.bfloat16)
    IDX = np.zeros((NCORES, 128, IC), dtype=np.int16)
    SW = np.zeros((NCORES, 128, TC, 128), dtype=sw_np_dt)
    FE = np.zeros((NCORES, 128, TC, D), dtype=ml_dtypes.bfloat16)
    for k in range(NCORES):
        o = order_per_core[k]
        e_rowc = src_rowc[o]
        e_src = src[o]
        e_dl = dst_p[o]
        e_w = edge_w[o].astype(np.float32)

        # slot positions: group (w,s) occupies tiles [ot|spos, +T), slot j ->
        # tile j//128, partition j%128
        grp_off = np.concatenate([[0], np.cumsum(cnt[k])]).astype(np.int64)
        g_sorted = grp[o]
        j_in_grp = np.arange(len(o)) - grp_off[g_sorted]
        gw = g_sorted // 4
        gs = g_sorted % 4
        tile_wm = ot_ws[gw, gs] + j_in_grp // PW
        tile_sm = spos_ws[gw, gs] + j_in_grp // PW
        part = j_in_grp % PW

        # idx stream (stream-major): real edges, then 0-pads up to the group's
        # cnt_max, then -1 (skipped when trailing in a gather) up to tile cap
        for s in range(4):
            m = gs == s
            vals = np.zeros(int(Ts[s]) * PW, dtype=np.int16)
            for w in range(WPC):
                cm = int(cnt_max[w * 4 + s])
                cap = int(Tws[w, s]) * PW
                sp0 = int(spos_ws[w, s]) * PW
                vals[sp0 + cm:sp0 + cap] = -1
            vals[(tile_sm[m] * PW + part[m])] = e_rowc[m].astype(np.int16)
            IDX[k, :, soff_idxcols[s]:soff_idxcols[s + 1]] = _wrap_idx(vals)

        # S_w (window-major): [part, tile, dst_loc] = edge weight
        SW[k, part, tile_wm, e_dl] = e_w.astype(sw_np_dt)
        # layer-0 feature stream (window-major edge order)
        FE[k, part, tile_wm, :] = featbf[e_src]

    # labels / mask per (core, partition, window)
    LBL = np.zeros((NCORES, 128, WPC), dtype=np.float32)
    MASK = np.zeros((NCORES, 128, WPC), dtype=np.float32)
    nn = np.arange(N)
    kk = nn // RPC
    off = nn % RPC
    LBL[kk, off % PW, off // PW] = labels.astype(np.float32)
    MASK[kk, off % PW, off // PW] = 1.0

    W1b = W1.astype(ml_dtypes.bfloat16)
    W2b = W2.astype(ml_dtypes.bfloat16)
    W3b = W3.astype(ml_dtypes.bfloat16)
    B1b = b1.reshape(1, -1).astype(ml_dtypes.bfloat16)
    B2b = b2.reshape(1, -1).astype(ml_dtypes.bfloat16)
    B3b = b3.reshape(1, -1).astype(ml_dtypes.bfloat16)

    # ---------------- bass program ----------------
    nc = bacc.Bacc("TRN2", target_bir_lowering=False, debug=False,
                   num_devices=NCORES, num_swdge_queues=4)

    sw_dt = mybir.dt.float8e4 if sw_fp8 else bf16
    fe_t = nc.dram_tensor("fe", [128, TC, D], bf16, kind="ExternalInput")
    sw_t = nc.dram_tensor("sw", [128, TC, 128], sw_dt, kind="ExternalInput")
    idx_t = nc.dram_tensor("idx", [128, IC], i16, kind="ExternalInput")
    lbl_t = nc.dram_tensor("lbl", [128, WPC], f32, kind="ExternalInput")
    mask_t = nc.dram_tensor("mask", [128, WPC], f32, kind="ExternalInput")
    w1_t = nc.dram_tensor("w1", [D, D], bf16, kind="ExternalInput")
    w2_t = nc.dram_tensor("w2", [D, D], bf16, kind="ExternalInput")
    w3_t = nc.dram_tensor("w3", [D, C], bf16, kind="ExternalInput")
    b1_t = nc.dram_tensor("bb1", [1, D], bf16, kind="ExternalInput")
    b2_t = nc.dram_tensor("bb2", [1, D], bf16, kind="ExternalInput")
    b3_t = nc.dram_tensor("bb3", [1, C], bf16, kind="ExternalInput")
    out_t = nc.dram_tensor("out", [1, 1], f32, kind="ExternalOutput")
    dump_layer = int(os.environ.get("GCN_DUMP", "-1"))
    hd_t = None
    if dump_layer >= 0:
        hd_t = [nc.dram_tensor(f"hd{c}", [CH_W[c] * PW, D], bf16,
                               kind="ExternalOutput") for c in range(4)]

    with tile.TileContext(nc) as tc:
        with (
            tc.tile_pool(name="const", bufs=1) as cpool,
            tc.tile_pool(name="small", bufs=3) as spool,
            tc.tile_pool(name="nll", bufs=2) as npool,
            tc.tile_pool(name="ps_agg", bufs=2, space="PSUM") as ps_agg,
            tc.tile_pool(name="ps_h", bufs=2, space="PSUM") as ps_h,
            tc.tile_pool(name="dram", bufs=1, space="DRAM") as dram,
            tc.tile_pool(name="ge0", bufs=2) as ge0,
            tc.tile_pool(name="ge1", bufs=2) as ge1,
            tc.tile_pool(name="ge2", bufs=2) as ge2,
            tc.tile_pool(name="ge3", bufs=2) as ge3,
            tc.tile_pool(name="go0", bufs=2) as go0,
            tc.tile_pool(name="go1", bufs=2) as go1,
            tc.tile_pool(name="go2", bufs=2) as go2,
            tc.tile_pool(name="go3", bufs=2) as go3,
            tc.tile_pool(name="swp", bufs=KSW) as swpool,
        ):
            gpool_ev = [ge0, ge1, ge2, ge3]
            gpool_od = [go0, go1, go2, go3]
            for s_ in range(4):
                for b_ in range(2):
                    for pool_, pfx_ in ((gpool_ev[s_], "ge"),
                                        (gpool_od[s_], "go")):
                        tz = pool_.tile([128, CHUNK_T, D], bf16,
                                        tag=f"{pfx_}{s_}", name=f"z{pfx_}{s_}{b_}")
                        nc.vector.memset(tz[:], 0.0)
            # ---- resident metadata ----
            idx_s = cpool.tile([128, IC], i16)
            lbl_s = cpool.tile([128, WPC], f32)
            mask_s = cpool.tile([128, WPC], f32)
            nc.sync.dma_start(out=idx_s[:], in_=idx_t[:])
            nc.sync.dma_start(out=lbl_s[:], in_=lbl_t[:])
            nc.sync.dma_start(out=mask_s[:], in_=mask_t[:])
            w_s = [cpool.tile([D, D], bf16, tag="w1", name="w1s"),
                   cpool.tile([D, D], bf16, tag="w2", name="w2s"),
                   cpool.tile([D, C], bf16, tag="w3", name="w3s")]
            nc.sync.dma_start(out=w_s[0][:], in_=w1_t[:])
            nc.sync.dma_start(out=w_s[1][:], in_=w2_t[:])
            nc.sync.dma_start(out=w_s[2][:], in_=w3_t[:])
            b_s = [cpool.tile([1, D], bf16, tag="b1", name="b1s"),
                   cpool.tile([1, D], bf16, tag="b2", name="b2s"),
                   cpool.tile([1, C], bf16, tag="b3", name="b3s")]
            nc.sync.dma_start(out=b_s[0][:], in_=b1_t[:])
            nc.sync.dma_start(out=b_s[1][:], in_=b2_t[:])
            nc.sync.dma_start(out=b_s[2][:], in_=b3_t[:])

            iota40 = cpool.tile([128, C], f32)
            nc.gpsimd.iota(iota40[:], pattern=[[1, C]], base=0,
                           channel_multiplier=0,
                           allow_small_or_imprecise_dtypes=True)
            ones1 = cpool.tile([1, 128], bf16)
            nc.vector.memset(ones1[:], 1.0)
            onescol = cpool.tile([128, 1], f32)
            nc.vector.memset(onescol[:], 1.0)
            nll_acc = cpool.tile([128, 1], f32)
            nc.vector.memset(nll_acc[:], 0.0)

            # stream tiles are allocated from rotating pools per chunk (the
            # pool rotation is what gives WAR protection); these dicts map
            # chunk index -> live tile object for the consumers.
            sw_tiles = {}
            fe_tiles = {}
            g_tiles = [{}, {}, {}, {}]
            g_tiles_next = [{}, {}, {}, {}]

            # ---- inter-layer DRAM tables ----
            h_mine = [[dram.tile([CH_W[c] * PW, D], bf16, tag=f"hm{l}{c}",
                                 name=f"hm{l}{c}")
                       for c in range(4)] for l in range(2)]
            h_full = [[dram.tile([CH_ROWS[c], D], bf16, tag=f"hf{l}{c}",
                                 name=f"hf{l}{c}", addr_space="Shared")
                       for c in range(4)] for l in range(2)]

            qcounter = [0]

            def issue_sw_chunk(c):
                T = int(min(CHUNK_T, TC - c * CHUNK_T))
                t_ = swpool.tile([128, CHUNK_T, 128], sw_dt, tag="sw",
                                 name=f"sw{c}")
                sw_tiles[c] = t_
                nc.sync.dma_start(out=t_[:, :T, :],
                                  in_=sw_t[:, c * CHUNK_T:c * CHUNK_T + T, :])

            def issue_fe_chunk(c):
                T = int(min(CHUNK_T, TC - c * CHUNK_T))
                s = c % 4
                t_ = gpool_ev[s].tile([128, CHUNK_T, D], bf16, tag=f"ge{s}",
                                      name=f"fe{c}")
                fe_tiles[c] = t_
                nc.sync.dma_start(out=t_[:, :T, :],
                                  in_=fe_t[:, c * CHUNK_T:c * CHUNK_T + T, :])

            MAXT_G = int(os.environ.get("GCN_MAXT", "8"))
            # per-stream list of (group start tile, end tile, cnt_max) in
            # stream-major order; gathers are issued per group segment so the
            # trailing -1 idx pads are actually skipped by the ucode
            grp_segs = [[] for _ in range(4)]
            for s_ in range(4):
                for w_ in range(WPC):
                    T_ = int(Tws[w_, s_])
                    if T_ == 0:
                        continue
                    g0 = int(spos_ws[w_, s_])
                    grp_segs[s_].append((g0, g0 + T_, int(cnt_max[w_ * 4 + s_])))

            def issue_gather_chunk(layer, s, c, into=None):
                lo, hi = c * CHUNK_T, min((c + 1) * CHUNK_T, int(Ts[s]))
                pool = gpool_od[s] if layer % 2 == 1 else gpool_ev[s]
                pfx = "go" if layer % 2 == 1 else "ge"
                t_ = pool.tile([128, CHUNK_T, D], bf16, tag=f"{pfx}{s}",
                               name=f"ga{layer}_{s}_{c}")
                (g_tiles if into is None else into)[s][c] = t_
                for (g0, g1, cm) in grp_segs[s]:
                    if g1 <= lo or g0 >= hi:
                        continue
                    a, b = max(g0, lo), min(g1, hi)
                    while a < b:
                        e = min(a + MAXT_G, b)
                        nvalid = max(0, min(cm - (a - g0) * PW, (e - a) * PW))
                        cb = int(soff_idxcols[s]) + a * 8
                        nc.gpsimd.dma_gather(
                            t_[:, a - lo:e - lo, :],
                            h_full[layer - 1][s][:],
                            idx_s[:, cb:cb + (e - a) * 8],
                            (e - a) * PW, nvalid, D,
                            queue_num=qcounter[0] % 4,
                        )
                        qcounter[0] += 1
                        a = e

            rg = [list(range(NCORES))]
            dbg = os.environ.get("GCN_DEBUG", "")
            n_layers = {"L1": 1, "L12": 2}.get(dbg, 3)

            PF = 2                 # prefetched chunks per stream
            for layer in range(n_layers):
                swc = [0]
                fec = [0]
                n_sw_chunks = -(-TC // CHUNK_T)
                sw_tiles.clear()
                for s_ in range(4):
                    g_tiles[s_].clear()
                    g_tiles[s_].update(g_tiles_next[s_])
                    g_tiles_next[s_] = {}
                gc = [len(g_tiles[s_]) for s_ in range(4)]

                def issue_ag(c):
                    nc.gpsimd.collective_compute(
                        "AllGather", mybir.AluOpType.bypass,
                        replica_groups=rg,
                        ins=[h_mine[layer][c].opt()],
                        outs=[h_full[layer][c].opt()],
                    )

                for w in range(WPC):
                    if layer < n_layers - 1 and layer < 2:
                        for c_ in range(4):
                            if w == CH_W0[c_] + CH_W[c_] - 1 + AG_DELAY:
                                issue_ag(c_)
                            # prefetch next layer's first chunks of stream c_
                            # once its AllGather has had time to complete

                    wl = min(w + LA, WPC - 1)
                    target_wm = int(ot_end_w[wl])
                    while swc[0] * CHUNK_T < target_wm and swc[0] < n_sw_chunks:
                        issue_sw_chunk(swc[0]); swc[0] += 1
                    if layer == 0:
                        while fec[0] * CHUNK_T < target_wm and fec[0] < n_sw_chunks:
                            issue_fe_chunk(fec[0]); fec[0] += 1
                    else:
                        for s in range(4):
                            tgt = int(spos_end_w[wl, s])
                            nmax = -(-int(Ts[s]) // CHUNK_T)
                            while gc[s] * CHUNK_T < tgt and gc[s] < nmax:
                                issue_gather_chunk(layer, s, gc[s]); gc[s] += 1

                    # ---- SpMM for window w ----
                    tiles = [(s, t) for s in range(4)
                             for t in range(int(Tws[w, s]))]
                    agg = ps_agg.tile([128, 128], f32)
                    for i, (s, t) in enumerate(tiles):
                        wm = int(ot_ws[w, s]) + t
                        if layer == 0:
                            gap = fe_tiles[wm // CHUNK_T][:, wm % CHUNK_T, :]
                        else:
                            sp = int(spos_ws[w, s]) + t
                            gap = g_tiles[s][sp // CHUNK_T][:, sp % CHUNK_T, :]
                        swap = sw_tiles[wm // CHUNK_T][:, wm % CHUNK_T, :]
                        nc.tensor.matmul(out=agg[:], lhsT=gap, rhs=swap,
                                         start=(i == 0),
                                         stop=(i == len(tiles) - 1))

                    aggT_sb = spool.tile([128, 128], bf16, tag="aggT")
                    nc.scalar.copy(aggT_sb[:], agg[:])
                    Dout = C if layer == 2 else D
                    ph = ps_h.tile([128, Dout], f32)
                    nc.tensor.matmul(out=ph[:], lhsT=aggT_sb[:],
                                     rhs=w_s[layer][:], start=True, stop=False)
                    nc.tensor.matmul(out=ph[:], lhsT=ones1[:],
                                     rhs=b_s[layer][:], start=False, stop=True)
                    if layer < 2:
                        ht = spool.tile([128, D], bf16, tag="ht")
                        nc.scalar.activation(ht[:], ph[:],
                                             mybir.ActivationFunctionType.Relu)
                        c = int(CHUNK_OF_W[w])
                        r0 = (w - CH_W0[c]) * PW
                        nc.sync.dma_start(out=h_mine[layer][c][r0:r0 + PW, :],
                                          in_=ht[:])
                        if layer == dump_layer:
                            nc.sync.dma_start(out=hd_t[c][r0:r0 + PW, :],
                                              in_=ht[:])

                    else:
                        # fused masked-NLL tail (f32)
                        mx = npool.tile([128, 1], f32, tag="mx")
                        nc.vector.tensor_reduce(out=mx[:], in_=ph[:],
                                                axis=mybir.AxisListType.X,
                                                op=mybir.AluOpType.max)
                        negmx = npool.tile([128, 1], f32, tag="negmx")
                        nc.vector.tensor_scalar_mul(negmx[:], mx[:], -1.0)
                        expb = npool.tile([128, C], f32, tag="expb")
                        sumexp = npool.tile([128, 1], f32, tag="sumexp")
                        nc.scalar.activation(expb[:], ph[:],
                                             mybir.ActivationFunctionType.Exp,
                                             bias=negmx[:, 0:1],
                                             accum_out=sumexp[:])
                        lse = npool.tile([128, 1], f32, tag="lse")
                        nc.scalar.activation(lse[:], sumexp[:],
                                             mybir.ActivationFunctionType.Ln)
                        junk = npool.tile([128, C], f32, tag="junk")
                        picked = npool.tile([128, 1], f32, tag="picked")
                        nc.vector.scalar_tensor_tensor(
                            out=junk[:], in0=iota40[:],
                            scalar=lbl_s[:, w:w + 1],
                            in1=ph[:],
                            op0=mybir.AluOpType.is_equal,
                            op1=mybir.AluOpType.mult,
                            accum_out=picked[:])
                        t1 = npool.tile([128, 1], f32, tag="t1")
                        nc.vector.tensor_tensor(out=t1[:], in0=lse[:],
                                                in1=negmx[:],
                                                op=mybir.AluOpType.subtract)
                        t2 = npool.tile([128, 1], f32, tag="t2")
                        nc.vector.tensor_tensor(out=t2[:], in0=t1[:],
                                                in1=picked[:],
                                                op=mybir.AluOpType.subtract)
                        nc.vector.scalar_tensor_tensor(
                            out=nll_acc[:], in0=t2[:],
                            scalar=mask_s[:, w:w + 1],
                            in1=nll_acc[:],
                            op0=mybir.AluOpType.mult,
                            op1=mybir.AluOpType.add)

                if layer < n_layers - 1 and layer < 2:
                    for c_ in range(4):
                        if CH_W0[c_] + CH_W[c_] - 1 + AG_DELAY > WPC - 1:
                            issue_ag(c_)
                    for s_ in range(4):
                        nmax_ = -(-int(Ts[s_]) // CHUNK_T)
                        for cpf in range(min(PF, nmax_)):
                            if cpf not in g_tiles_next[s_]:
                                issue_gather_chunk(layer + 1, s_, cpf,
                                                   into=g_tiles_next)

            # ---------------- final partial-sum ----------------
            pscalar = ps_h.tile([1, 1], f32, tag="pscalar")
            nc.tensor.matmul(out=pscalar[:], lhsT=nll_acc[:], rhs=onescol[:],
                             start=True, stop=True)
            res_sb = spool.tile([1, 1], f32, tag="res")
            nc.scalar.copy(res_sb[:], pscalar[:])
            nc.sync.dma_start(out=out_t[:], in_=res_sb[:])

    nc.compile()

    in_maps = []
    for k in range(NCORES):
        in_maps.append({
            "fe": FE[k], "sw": SW[k], "idx": IDX[k],
            "lbl": LBL[k], "mask": MASK[k],
            "w1": W1b, "w2": W2b, "w3": W3b,
            "bb1": B1b, "bb2": B2b, "bb3": B3b,
        })
    trace_ok = False
    try:
        from antenv.axon_hooks import get_axon_ntff_profile_hook
        trace_ok = get_axon_ntff_profile_hook() is not None
    except Exception:
        pass
    res = run_bass_kernel_spmd(nc, in_maps, list(range(NCORES)), trace=trace_ok)
    global LAST_EXEC_NS, LAST_RESULT
    LAST_EXEC_NS = res.exec_time_ns
    LAST_RESULT = res
    total = sum(float(res.results[k]["out"][0, 0]) for k in range(NCORES))
    return np.float32(total / N)
